# revision 41
# baseline (speedup 1.0000x reference)
"""GQA attention kernel for 8 Trainium2 NeuronCores.

Sharding: core = (batch b, kv_group g), b in {0,1}, g in {0..3}.
Each core computes the 4 heads of one KV group for one batch and the
partial output projection for those heads; the host sums the 4 group
partials per batch.  Zero duplicated compute across cores.

v4 design (baseline was 516us):
  - P1 (QKV proj) e-outer with 6 concurrent PSUM accumulation groups;
    e-granular first-quarter DMAs spread across the SP/ACT/Pool DGE
    queues so the first matmul starts ~12us in and PE never waits on
    DMA; V transposed through a side PSUM bank inside the loop.
  - P2 (attention) software-pipelined with lookahead-2 scores in a
    3-deep PSUM rotation so PE never stalls on the ACT exp chain.
  - softmax denominators: probs written bf16; accumulation split
    across DVE (5 tiles + init copy), Pool (7 tiles), and PE
    (3 tail tiles via the final ones-matmul), sized from measured
    per-op costs so every engine stays under PE's per-block time.
  - per-block normalize (reciprocal + multiply) deferred 4 tiles into
    the next block, AV PSUM drained immediately by DVE, so no PE
    instruction ever waits on the normalize chain; the last block's
    reduction uses the AV PSUM pool so phase 3's PSUM pool opens
    without waiting on it.
  - numerics: scores/Q/K/weights stay fp32r; only probs/V/acc are
    bf16 (validated 2.1e-3 max rel err vs 2e-2 budget).
"""

import numpy as np

# problem shape (hardcoded per contract)
B, S, E = 2, 2048, 2048
H, G, D = 16, 4, 128
R = H // G          # heads per kv group = 4
KV = G * D          # 512
ST = S // 128       # 16 t-tiles
ET = E // 128       # 16 e-tiles
SC = S // 512       # 4 s-chunks
NPAIR = S // 1024   # 2 q-chunk pairs
NBLK = R * NPAIR    # 8 attention blocks per core
LOOK = 2            # scores lookahead (PSUM rotation depth - 1)

_cache = {}


def _split_multi_waits(nc, maxw=1):
    """Walrus in this container accepts only one sync-wait per
    instruction; move extra waits onto preceding same-engine NoOps."""
    from concourse import mybir

    n_split = 0
    for fn in nc.m.functions:
        for bb in fn.blocks:
            out = []
            changed = False
            for inst in bb.instructions:
                si = inst.sync_info
                waits = list(si.on_wait or []) if si is not None else []
                if len(waits) > maxw:
                    changed = True
                    n_split += 1
                    head, tail = waits[:-maxw], waits[-maxw:]
                    for j in range(0, len(head), maxw):
                        nop = mybir.InstNoOp(
                            name=f"{inst.name}-wsplit{j}", ins=[], outs=[]
                        )
                        nop.engine = inst.engine
                        nop.sync_info = mybir.SyncInfo(
                            on_wait=head[j : j + maxw], on_update=[]
                        )
                        out.append(nop)
                    si.on_wait = tail
                out.append(inst)
            if changed:
                bb.instructions = out
    return n_split


def _build_program():
    import concourse.bass as bass
    import concourse.tile as tile
    from concourse import mybir
    from concourse.masks import make_identity

    F32R = mybir.dt.float32r
    F32 = mybir.dt.float32
    BF16 = mybir.dt.bfloat16
    Exp = mybir.ActivationFunctionType.Exp
    Mult = mybir.AluOpType.mult
    Add = mybir.AluOpType.add

    nc = bass.Bass(target_bir_lowering=False)

    # x and the QKV weights arrive as bf16 (host-cast): halves input DMA
    # bytes — the DGE rings are the phase-1 constraint — at identical
    # matmul throughput (1 row/cycle for bf16 and fp32r alike)
    xT = nc.dram_tensor("xT", [E, S], BF16, kind="ExternalInput")
    wq = nc.dram_tensor("wq", [E, R * D], BF16, kind="ExternalInput")
    wk = nc.dram_tensor("wk", [E, D], BF16, kind="ExternalInput")
    wv = nc.dram_tensor("wv", [E, D], BF16, kind="ExternalInput")
    wo = nc.dram_tensor("wo", [R * D, E], F32R, kind="ExternalInput")
    bqv = nc.dram_tensor("bqv", [R * D], F32, kind="ExternalInput")
    bkv = nc.dram_tensor("bkv", [D], F32, kind="ExternalInput")
    bvv = nc.dram_tensor("bvv", [D], F32, kind="ExternalInput")
    otd = nc.dram_tensor("ot", [E, S], F32, kind="ExternalOutput")

    xTr = xT.rearrange("(o p) m -> p o m", p=128)
    wqr = wq.rearrange("(o p) m -> p o m", p=128)
    wkr = wk.rearrange("(o p) m -> p o m", p=128)
    wvr = wv.rearrange("(o p) m -> p o m", p=128)
    wor = wo.rearrange("(o p) m -> p o m", p=128)

    with tile.TileContext(nc) as tc:
        import contextlib

        with contextlib.ExitStack() as ctx:
            consts = ctx.enter_context(tc.tile_pool(name="consts", bufs=1))
            qkvt = ctx.enter_context(tc.tile_pool(name="qkvt", bufs=1))

            ident_f = consts.tile([128, 128], F32)
            make_identity(nc, ident_f)
            ident = consts.tile([128, 128], F32R)
            nc.vector.tensor_copy(ident, ident_f)
            ones_bf = consts.tile([128, 128], BF16)
            nc.gpsimd.memset(ones_bf, 1.0)
            bq_sb = consts.tile([128, R], F32)
            bk_sb = consts.tile([128, 1], F32)
            bv_sb = consts.tile([128, 1], F32)

            QT = qkvt.tile([128, R, S], F32R)    # QT[d, h, s]
            KT = qkvt.tile([128, S], F32R)       # KT[d, t]
            V = qkvt.tile([128, ST, D], BF16)    # V[t%128, tt, d]

            # ---- phase 1: QKV^T projections + V transpose ----
            with tc.tile_pool(name="vt", bufs=1) as vtpool, \
                 tc.tile_pool(name="wts", bufs=1) as wpool, \
                 tc.tile_pool(name="xts", bufs=4) as xtpool, \
                 tc.tile_pool(name="ps1", bufs=7, space="PSUM") as ps1, \
                 tc.tile_pool(name="psv", bufs=1, space="PSUM") as psv:
                VT = vtpool.tile([128, S], F32R)
                wq_sb = wpool.tile([128, ET, R * D], BF16)
                wk_sb = wpool.tile([128, ET, D], BF16)
                wv_sb = wpool.tile([128, ET, D], BF16)
                # e-granular DMAs for the first quarter so the first
                # matmuls unblock asap; remaining quarters spread over the
                # SP and ACT DGE queues so neither queue serializes >3MB
                nc.sync.dma_start(wq_sb[:, 0:1], wqr[:, 0:1])
                nc.scalar.dma_start(wk_sb[:, 0:4], wkr[:, 0:4])
                nc.scalar.dma_start(wv_sb[:, 0:4], wvr[:, 0:4])
                nc.sync.dma_start(wq_sb[:, 1:4], wqr[:, 1:4])
                nc.sync.dma_start(wq_sb[:, 4:8], wqr[:, 4:8])
                nc.scalar.dma_start(wq_sb[:, 8:12], wqr[:, 8:12])
                nc.scalar.dma_start(wq_sb[:, 12:16], wqr[:, 12:16])
                for half in (slice(4, 10), slice(10, ET)):
                    nc.sync.dma_start(wk_sb[:, half], wkr[:, half])
                    nc.sync.dma_start(wv_sb[:, half], wvr[:, half])
                # biases are tiny and needed late; issue after the weights
                nc.sync.dma_start(bq_sb, bqv.rearrange("(o p) -> p o", p=128))
                nc.sync.dma_start(bk_sb, bkv.rearrange("(o p) -> p o", p=128))
                nc.sync.dma_start(bv_sb, bvv.rearrange("(o p) -> p o", p=128))

                def transposes(sc):
                    tps = psv.tile([128, 512], F32R, tag="pv", name="tps")
                    for i in range(4):
                        tt = sc * 4 + i
                        nc.tensor.transpose(
                            tps[:, i * 128 : (i + 1) * 128],
                            VT[:, tt * 128 : (tt + 1) * 128],
                            ident,
                        )
                    for i in range(4):
                        nc.vector.tensor_copy(
                            V[:, sc * 4 + i], tps[:, i * 128 : (i + 1) * 128]
                        )

                for sc in range(SC):
                    cs = slice(sc * 512, (sc + 1) * 512)
                    po = [ps1.tile([128, 512], F32, tag="p1", name="po")
                          for _ in range(R + 2)]
                    for eq in range(4):
                        xq = xtpool.tile([128, 4, 512], BF16, tag="xt")
                        if sc == 0 and eq == 0:
                            # e-granular so the first matmul starts early
                            for i in range(4):
                                nc.gpsimd.dma_start(
                                    xq[:, i : i + 1], xTr[:, i : i + 1, cs]
                                )
                        elif sc == 0:
                            # pair-granular through the rest of the first
                            # chunk: the ring can't stay ahead of PE with
                            # full quarters this early
                            for i in (0, 2):
                                e0 = eq * 4 + i
                                nc.gpsimd.dma_start(
                                    xq[:, i : i + 2], xTr[:, e0 : e0 + 2, cs]
                                )
                        else:
                            nc.gpsimd.dma_start(
                                xq, xTr[:, eq * 4 : eq * 4 + 4, cs]
                            )
                        for i in range(4):
                            e = eq * 4 + i
                            for ot in range(R + 2):
                                if ot < R:
                                    lhsT = wq_sb[:, e, ot * 128 : (ot + 1) * 128]
                                elif ot == R:
                                    lhsT = wk_sb[:, e]
                                else:
                                    lhsT = wv_sb[:, e]
                                nc.tensor.matmul(
                                    po[ot], lhsT, xq[:, i],
                                    start=(e == 0), stop=(e == ET - 1),
                                )
                        if eq == 1 and sc > 0:
                            # previous chunk's V rows are long since
                            # drained; transpose them here so PE never
                            # waits on the ACT drain queue
                            transposes(sc - 1)
                    # drains; for the last chunk emit V first so its
                    # transposes (right below) wait minimally
                    drains = [(VT[:, cs], po[R + 1], bv_sb[:, 0:1]),
                              (KT[:, cs], po[R], bk_sb[:, 0:1])]
                    qdr = [(QT[:, ot, cs], po[ot], bq_sb[:, ot : ot + 1])
                           for ot in range(R)]
                    order = drains + qdr if sc == SC - 1 else qdr + drains[::-1]
                    for dst, src, bias in order:
                        nc.scalar.add(dst, src, bias)
                transposes(SC - 1)

            # ---- phase 2: attention, software-pipelined ----
            p23 = ctx.enter_context(tc.tile_pool(name="p23", bufs=1))
            outT = p23.tile([128, R, S], F32R)  # normalized attn outT[d, h, s]
            wo_sb = p23.tile([128, R, E], F32R)
            for q in range(4):
                nc.sync.dma_start(wo_sb[:, q], wor[:, q])

            with tc.tile_pool(name="ps_av", bufs=1, space="PSUM") as ps_av, \
                 tc.tile_pool(name="probs", bufs=10) as probs_pool, \
                 tc.tile_pool(name="accp", bufs=2) as accp, \
                 tc.tile_pool(name="avsb", bufs=2) as avsb, \
                 tc.tile_pool(name="smsb", bufs=2) as smsb, \
                 tc.tile_pool(name="rcp", bufs=2) as rcp:

                pss_t = {}   # j -> scores psum tile
                acc_t = {}   # blk -> (dve_acc, pool_acc) bf16 accumulators
                av_t = {}    # blk -> AV psum tile
                avsb_t = {}  # blk -> AV sbuf drain tile
                pt_tail = {}  # blk -> tail prob tiles summed directly on PE

                def finalize(b, sums_pool, sums_tag):
                    """Deferred per-block tail: reduce the prob accumulators
                    (+ the tail prob tiles, summed directly on PE to offload
                    DVE/Pool), reciprocal, and normalize into outT.  Runs 5
                    tiles into the next block so nothing on PE ever waits
                    for it.  The sums PSUM tile is drained to SBUF by a fast
                    DVE copy before the slow reciprocal reads it, so the
                    PSUM slot recycles in ~0.7us instead of ~6.5us (the
                    reciprocal-blocks-scores WAR stall)."""
                    h, pr = b // NPAIR, b % NPAIR
                    q0 = pr * 1024
                    acc_d, acc_p = acc_t.pop(b)
                    srcs = [acc_d, acc_p] + pt_tail.pop(b)
                    sums = sums_pool.tile(
                        [128, 1024], F32, tag=sums_tag, name="sums"
                    )
                    for hf in range(2):
                        hs = slice(hf * 512, (hf + 1) * 512)
                        for si, src in enumerate(srcs):
                            nc.tensor.matmul(
                                sums[:, hs], ones_bf, src[:, hs],
                                start=(si == 0), stop=(si == len(srcs) - 1),
                            )
                    ssb = smsb.tile([128, 1024], F32, tag="ssb", name="ssb")
                    nc.vector.tensor_copy(ssb, sums)
                    rc = rcp.tile([128, 1024], F32, tag="rc", name="rc")
                    nc.vector.reciprocal(rc, ssb)
                    # normalize on DVE right after the reciprocal so phase 3
                    # never waits behind the next block's Pool adds
                    nc.vector.tensor_tensor(
                        outT[:, h, q0 : q0 + 1024], avsb_t.pop(b), rc, Mult
                    )

                with tc.tile_pool(name="ps_mix", bufs=3, space="PSUM") as ps_mix:
                    for j in range(16 * NBLK + LOOK):
                        if j < 16 * NBLK:
                            b, tt = j // ST, j % ST
                            h, pr = b // NPAIR, b % NPAIR
                            q0 = pr * 1024
                            pss = ps_mix.tile(
                                [128, 1024], F32, tag="s", name="pss"
                            )
                            kslice = KT[:, tt * 128 : (tt + 1) * 128]
                            for hf in range(2):
                                nc.tensor.matmul(
                                    pss[:, hf * 512 : (hf + 1) * 512],
                                    kslice,
                                    QT[:, h, q0 + hf * 512 : q0 + (hf + 1) * 512],
                                    start=True, stop=True,
                                )
                            pss_t[j] = pss
                        jj = j - LOOK
                        if 0 <= jj < 16 * NBLK:
                            b, tt = jj // ST, jj % ST
                            h, pr = b // NPAIR, b % NPAIR
                            q0 = pr * 1024
                            pt = probs_pool.tile(
                                [128, 1024], BF16, tag="pt", name="pt"
                            )
                            nc.scalar.activation(pt, pss_t.pop(jj), Exp)
                            # denominator accumulation split, sized from
                            # measured per-op costs so each engine stays
                            # under PE's per-block time: DVE gets the init
                            # copy + 4 even tiles, Pool 6 tiles, PE the last
                            # 5 via finalize's ones-matmul
                            if tt == 0:
                                acc_d = accp.tile(
                                    [128, 1024], BF16, tag="accd", name="accd"
                                )
                                acc_p = accp.tile(
                                    [128, 1024], BF16, tag="accp", name="accp"
                                )
                                acc_t[b] = (acc_d, acc_p)
                                pt_tail[b] = []
                                nc.vector.tensor_copy(acc_d, pt)
                                nc.gpsimd.memset(acc_p, 0.0)
                                av_t[b] = ps_av.tile(
                                    [128, 1024], F32, tag="av", name="avp"
                                )
                            elif tt >= (ST - 7 if b == NBLK - 1 else ST - 5):
                                # last block hands two extra tiles to PE:
                                # there is no following block to hide the
                                # Pool adds' latency behind
                                pt_tail[b].append(pt)
                            elif tt % 2 == 0 and tt <= 8:
                                acc_d = acc_t[b][0]
                                nc.vector.tensor_tensor(acc_d, acc_d, pt, Add)
                            else:
                                acc_p = acc_t[b][1]
                                nc.gpsimd.tensor_tensor(acc_p, acc_p, pt, Add)
                            av = av_t[b]
                            for hf in range(2):
                                hs = slice(hf * 512, (hf + 1) * 512)
                                nc.tensor.matmul(
                                    av[:, hs], V[:, tt], pt[:, hs],
                                    start=(tt == 0), stop=(tt == ST - 1),
                                )
                            if tt == ST - 1:
                                # drain AV psum immediately so the next
                                # block's AV start never waits on the
                                # normalize chain
                                asb = avsb.tile(
                                    [128, 1024], F32, tag="avsb", name="asb"
                                )
                                avsb_t[b] = asb
                                av = av_t.pop(b)
                                # split halves: the next block's first AV
                                # matmul (hf=0) unblocks after the first
                                # half-copy instead of the full drain
                                nc.vector.tensor_copy(
                                    asb[:, 0:512], av[:, 0:512]
                                )
                                nc.vector.tensor_copy(
                                    asb[:, 512:1024], av[:, 512:1024]
                                )
                        # per-block tail emitted right before the next
                        # block's first AV matmuls: the 14-matmul sums burst
                        # fills the window where those AVs would otherwise
                        # stall on the exp pipeline, and the exp queue
                        # catches up during the burst
                        if j >= 18 and (j - 18) % 16 == 0 and (j - 18) // 16 < NBLK - 1:
                            finalize((j - 18) // 16, ps_mix, "s")

                    # last block's reduction goes through the AV pool (not
                    # ps_mix), emitted before ps_mix closes so the close and
                    # phase 3's pool open overlap the final reciprocal
                    finalize(NBLK - 1, ps_av, "av")

                # ---- phase 3: output projection (transposed) ----
                otr = otd.rearrange("(o p) m -> p o m", p=128)
                with tc.tile_pool(name="ostage", bufs=4) as ostage, \
                     tc.tile_pool(name="ps_o", bufs=6, space="PSUM") as ps_o:
                    # sc outer: the first tiles only need the pr=0 (even)
                    # blocks, so P3 never waits on the last blocks' deferred
                    # normalize; output DMAs go out in et-pairs alternating
                    # between two DGE rings so the ring never backs up
                    for sc in range(SC):
                        cs = slice(sc * 512, (sc + 1) * 512)
                        for etp in range(ET // 2):
                            st = ostage.tile(
                                [128, 2, 512], F32, tag="ost", name="st"
                            )
                            for k in range(2):
                                et = etp * 2 + k
                                ps = ps_o.tile(
                                    [128, 512], F32, tag="po", name="ps"
                                )
                                for h in range(R):
                                    nc.tensor.matmul(
                                        ps,
                                        wo_sb[:, h, et * 128 : (et + 1) * 128],
                                        outT[:, h, cs],
                                        start=(h == 0), stop=(h == R - 1),
                                    )
                                nc.vector.tensor_copy(st[:, k], ps)
                            eng = (nc.sync, nc.scalar, nc.gpsimd)[etp % 3]
                            eng.dma_start(
                                otr[:, etp * 2 : etp * 2 + 2, cs], st
                            )

    _split_multi_waits(nc)
    return nc


def _prepare(x, Wq, bq, Wk, bk, Wv, bv, Wo, bo):
    """Host-side sharding: build per-core input maps."""
    x = np.asarray(x, dtype=np.float32)
    Wq = np.asarray(Wq, dtype=np.float32)
    bq = np.asarray(bq, dtype=np.float32)
    Wk = np.asarray(Wk, dtype=np.float32)
    bk = np.asarray(bk, dtype=np.float32)
    Wv = np.asarray(Wv, dtype=np.float32)
    bv = np.asarray(bv, dtype=np.float32)
    Wo = np.asarray(Wo, dtype=np.float32)

    import ml_dtypes

    BF = ml_dtypes.bfloat16
    isd = np.float32(1.0 / np.sqrt(D))
    xTs = [np.ascontiguousarray(x[b].T.astype(BF)) for b in range(B)]
    in_maps = []
    for core in range(8):
        b, g = divmod(core, G)
        in_maps.append({
            "xT": xTs[b],
            "wq": np.ascontiguousarray(
                (Wq[:, g * R * D : (g + 1) * R * D] * isd).astype(BF)
            ),
            "wk": np.ascontiguousarray(Wk[:, g * D : (g + 1) * D].astype(BF)),
            "wv": np.ascontiguousarray(Wv[:, g * D : (g + 1) * D].astype(BF)),
            "wo": np.ascontiguousarray(Wo[g * R * D : (g + 1) * R * D, :]),
            "bqv": bq[g * R * D : (g + 1) * R * D] * isd,
            "bkv": bk[g * D : (g + 1) * D],
            "bvv": bv[g * D : (g + 1) * D],
        })
    return in_maps


def _gather(results, bo):
    bo = np.asarray(bo, dtype=np.float32)
    out = np.empty((B, S, E), dtype=np.float32)
    for b in range(B):
        acc = results[b * G]["ot"].copy()
        for g in range(1, G):
            acc += results[b * G + g]["ot"]
        out[b] = acc.T + bo
    return out


def kernel(x, Wq, bq, Wk, bk, Wv, bv, Wo, bo):
    from concourse.bass_utils import run_bass_kernel_spmd

    if "nc" not in _cache:
        _cache["nc"] = _build_program()
    nc = _cache["nc"]
    in_maps = _prepare(x, Wq, bq, Wk, bk, Wv, bv, Wo, bo)
    res = run_bass_kernel_spmd(nc, in_maps, core_ids=list(range(8)))
    return _gather(res.results, bo)


# revision 42
# speedup vs baseline: 1.0025x; 1.0025x over previous
"""GQA attention kernel for 8 Trainium2 NeuronCores.

Sharding: core = (batch b, kv_group g), b in {0,1}, g in {0..3}.
Each core computes the 4 heads of one KV group for one batch and the
partial output projection for those heads; the host sums the 4 group
partials per batch.  Zero duplicated compute across cores.

v4 design (baseline was 516us):
  - P1 (QKV proj) e-outer with 6 concurrent PSUM accumulation groups;
    e-granular first-quarter DMAs spread across the SP/ACT/Pool DGE
    queues so the first matmul starts ~12us in and PE never waits on
    DMA; V transposed through a side PSUM bank inside the loop.
  - P2 (attention) software-pipelined with lookahead-2 scores in a
    3-deep PSUM rotation so PE never stalls on the ACT exp chain.
  - softmax denominators: probs written bf16; accumulation split
    across DVE (5 tiles + init copy), Pool (7 tiles), and PE
    (3 tail tiles via the final ones-matmul), sized from measured
    per-op costs so every engine stays under PE's per-block time.
  - per-block normalize (reciprocal + multiply) deferred 4 tiles into
    the next block, AV PSUM drained immediately by DVE, so no PE
    instruction ever waits on the normalize chain; the last block's
    reduction uses the AV PSUM pool so phase 3's PSUM pool opens
    without waiting on it.
  - numerics: scores/Q/K/weights stay fp32r; only probs/V/acc are
    bf16 (validated 2.1e-3 max rel err vs 2e-2 budget).
"""

import numpy as np

# problem shape (hardcoded per contract)
B, S, E = 2, 2048, 2048
H, G, D = 16, 4, 128
R = H // G          # heads per kv group = 4
KV = G * D          # 512
ST = S // 128       # 16 t-tiles
ET = E // 128       # 16 e-tiles
SC = S // 512       # 4 s-chunks
NPAIR = S // 1024   # 2 q-chunk pairs
NBLK = R * NPAIR    # 8 attention blocks per core
LOOK = 2            # scores lookahead (PSUM rotation depth - 1)

_cache = {}


def _split_multi_waits(nc, maxw=1):
    """Walrus in this container accepts only one sync-wait per
    instruction; move extra waits onto preceding same-engine NoOps."""
    from concourse import mybir

    n_split = 0
    for fn in nc.m.functions:
        for bb in fn.blocks:
            out = []
            changed = False
            for inst in bb.instructions:
                si = inst.sync_info
                waits = list(si.on_wait or []) if si is not None else []
                if len(waits) > maxw:
                    changed = True
                    n_split += 1
                    head, tail = waits[:-maxw], waits[-maxw:]
                    for j in range(0, len(head), maxw):
                        nop = mybir.InstNoOp(
                            name=f"{inst.name}-wsplit{j}", ins=[], outs=[]
                        )
                        nop.engine = inst.engine
                        nop.sync_info = mybir.SyncInfo(
                            on_wait=head[j : j + maxw], on_update=[]
                        )
                        out.append(nop)
                    si.on_wait = tail
                out.append(inst)
            if changed:
                bb.instructions = out
    return n_split


def _build_program():
    import concourse.bass as bass
    import concourse.tile as tile
    from concourse import mybir
    from concourse.masks import make_identity

    F32R = mybir.dt.float32r
    F32 = mybir.dt.float32
    BF16 = mybir.dt.bfloat16
    Exp = mybir.ActivationFunctionType.Exp
    Mult = mybir.AluOpType.mult
    Add = mybir.AluOpType.add

    nc = bass.Bass(target_bir_lowering=False)

    # x and the QKV weights arrive as bf16 (host-cast): halves input DMA
    # bytes — the DGE rings are the phase-1 constraint — at identical
    # matmul throughput (1 row/cycle for bf16 and fp32r alike)
    xT = nc.dram_tensor("xT", [E, S], BF16, kind="ExternalInput")
    wq = nc.dram_tensor("wq", [E, R * D], BF16, kind="ExternalInput")
    wk = nc.dram_tensor("wk", [E, D], BF16, kind="ExternalInput")
    wv = nc.dram_tensor("wv", [E, D], BF16, kind="ExternalInput")
    wo = nc.dram_tensor("wo", [R * D, E], F32R, kind="ExternalInput")
    bqv = nc.dram_tensor("bqv", [R * D], F32, kind="ExternalInput")
    bkv = nc.dram_tensor("bkv", [D], F32, kind="ExternalInput")
    bvv = nc.dram_tensor("bvv", [D], F32, kind="ExternalInput")
    otd = nc.dram_tensor("ot", [E, S], F32, kind="ExternalOutput")

    xTr = xT.rearrange("(o p) m -> p o m", p=128)
    wqr = wq.rearrange("(o p) m -> p o m", p=128)
    wkr = wk.rearrange("(o p) m -> p o m", p=128)
    wvr = wv.rearrange("(o p) m -> p o m", p=128)
    wor = wo.rearrange("(o p) m -> p o m", p=128)

    with tile.TileContext(nc) as tc:
        import contextlib

        with contextlib.ExitStack() as ctx:
            consts = ctx.enter_context(tc.tile_pool(name="consts", bufs=1))
            qkvt = ctx.enter_context(tc.tile_pool(name="qkvt", bufs=1))

            ident_f = consts.tile([128, 128], F32)
            make_identity(nc, ident_f)
            ident = consts.tile([128, 128], F32R)
            nc.vector.tensor_copy(ident, ident_f)
            ones_bf = consts.tile([128, 128], BF16)
            nc.gpsimd.memset(ones_bf, 1.0)
            bq_sb = consts.tile([128, R], F32)
            bk_sb = consts.tile([128, 1], F32)
            bv_sb = consts.tile([128, 1], F32)

            QT = qkvt.tile([128, R, S], F32R)    # QT[d, h, s]
            KT = qkvt.tile([128, S], F32R)       # KT[d, t]
            V = qkvt.tile([128, ST, D], BF16)    # V[t%128, tt, d]

            # ---- phase 1: QKV^T projections + V transpose ----
            with tc.tile_pool(name="vt", bufs=1) as vtpool, \
                 tc.tile_pool(name="wts", bufs=1) as wpool, \
                 tc.tile_pool(name="xts", bufs=4) as xtpool, \
                 tc.tile_pool(name="ps1", bufs=7, space="PSUM") as ps1, \
                 tc.tile_pool(name="psv", bufs=1, space="PSUM") as psv:
                VT = vtpool.tile([128, S], F32R)
                wq_sb = wpool.tile([128, ET, R * D], BF16)
                wk_sb = wpool.tile([128, ET, D], BF16)
                wv_sb = wpool.tile([128, ET, D], BF16)
                # e-granular DMAs for the first quarter so the first
                # matmuls unblock asap; remaining quarters spread over the
                # SP and ACT DGE queues so neither queue serializes >3MB
                nc.sync.dma_start(wq_sb[:, 0:1], wqr[:, 0:1])
                nc.scalar.dma_start(wk_sb[:, 0:4], wkr[:, 0:4])
                nc.scalar.dma_start(wv_sb[:, 0:4], wvr[:, 0:4])
                nc.sync.dma_start(wq_sb[:, 1:4], wqr[:, 1:4])
                nc.sync.dma_start(wq_sb[:, 4:8], wqr[:, 4:8])
                nc.scalar.dma_start(wq_sb[:, 8:12], wqr[:, 8:12])
                nc.scalar.dma_start(wq_sb[:, 12:16], wqr[:, 12:16])
                for half in (slice(4, 10), slice(10, ET)):
                    nc.sync.dma_start(wk_sb[:, half], wkr[:, half])
                    nc.sync.dma_start(wv_sb[:, half], wvr[:, half])
                # biases are tiny and needed late; issue after the weights
                nc.sync.dma_start(bq_sb, bqv.rearrange("(o p) -> p o", p=128))
                nc.sync.dma_start(bk_sb, bkv.rearrange("(o p) -> p o", p=128))
                nc.sync.dma_start(bv_sb, bvv.rearrange("(o p) -> p o", p=128))

                def transposes(sc):
                    tps = psv.tile([128, 512], F32R, tag="pv", name="tps")
                    for i in range(4):
                        tt = sc * 4 + i
                        nc.tensor.transpose(
                            tps[:, i * 128 : (i + 1) * 128],
                            VT[:, tt * 128 : (tt + 1) * 128],
                            ident,
                        )
                    for i in range(4):
                        nc.vector.tensor_copy(
                            V[:, sc * 4 + i], tps[:, i * 128 : (i + 1) * 128]
                        )

                for sc in range(SC):
                    cs = slice(sc * 512, (sc + 1) * 512)
                    po = [ps1.tile([128, 512], F32, tag="p1", name="po")
                          for _ in range(R + 2)]
                    for eq in range(4):
                        xq = xtpool.tile([128, 4, 512], BF16, tag="xt")
                        if sc == 0 and eq == 0:
                            # e-granular so the first matmul starts early
                            for i in range(4):
                                nc.gpsimd.dma_start(
                                    xq[:, i : i + 1], xTr[:, i : i + 1, cs]
                                )
                        elif sc == 0:
                            # pair-granular through the rest of the first
                            # chunk: the ring can't stay ahead of PE with
                            # full quarters this early
                            for i in (0, 2):
                                e0 = eq * 4 + i
                                nc.gpsimd.dma_start(
                                    xq[:, i : i + 2], xTr[:, e0 : e0 + 2, cs]
                                )
                        else:
                            nc.gpsimd.dma_start(
                                xq, xTr[:, eq * 4 : eq * 4 + 4, cs]
                            )
                        for i in range(4):
                            e = eq * 4 + i
                            for ot in range(R + 2):
                                if ot < R:
                                    lhsT = wq_sb[:, e, ot * 128 : (ot + 1) * 128]
                                elif ot == R:
                                    lhsT = wk_sb[:, e]
                                else:
                                    lhsT = wv_sb[:, e]
                                nc.tensor.matmul(
                                    po[ot], lhsT, xq[:, i],
                                    start=(e == 0), stop=(e == ET - 1),
                                )
                        if eq == 1 and sc > 0:
                            # previous chunk's V rows are long since
                            # drained; transpose them here so PE never
                            # waits on the ACT drain queue
                            transposes(sc - 1)
                    # drains; for the last chunk emit V first so its
                    # transposes (right below) wait minimally
                    drains = [(VT[:, cs], po[R + 1], bv_sb[:, 0:1]),
                              (KT[:, cs], po[R], bk_sb[:, 0:1])]
                    qdr = [(QT[:, ot, cs], po[ot], bq_sb[:, ot : ot + 1])
                           for ot in range(R)]
                    order = drains + qdr if sc == SC - 1 else qdr + drains[::-1]
                    for dst, src, bias in order:
                        nc.scalar.add(dst, src, bias)
                transposes(SC - 1)

            # ---- phase 2: attention, software-pipelined ----
            p23 = ctx.enter_context(tc.tile_pool(name="p23", bufs=1))
            outT = p23.tile([128, R, S], F32R)  # normalized attn outT[d, h, s]
            wo_sb = p23.tile([128, R, E], F32R)
            for q in range(4):
                nc.sync.dma_start(wo_sb[:, q], wor[:, q])

            with tc.tile_pool(name="ps_av", bufs=1, space="PSUM") as ps_av, \
                 tc.tile_pool(name="probs", bufs=10) as probs_pool, \
                 tc.tile_pool(name="accp", bufs=2) as accp, \
                 tc.tile_pool(name="avsb", bufs=2) as avsb, \
                 tc.tile_pool(name="smsb", bufs=2) as smsb, \
                 tc.tile_pool(name="rcp", bufs=2) as rcp:

                pss_t = {}   # j -> scores psum tile
                acc_t = {}   # blk -> (dve_acc, pool_acc) bf16 accumulators
                av_t = {}    # blk -> AV psum tile
                avsb_t = {}  # blk -> AV sbuf drain tile
                pt_tail = {}  # blk -> tail prob tiles summed directly on PE

                def finalize(b, sums_pool, sums_tag):
                    """Deferred per-block tail: reduce the prob accumulators
                    (+ the tail prob tiles, summed directly on PE to offload
                    DVE/Pool), reciprocal, and normalize into outT.  Runs 5
                    tiles into the next block so nothing on PE ever waits
                    for it.  The sums PSUM tile is drained to SBUF by a fast
                    DVE copy before the slow reciprocal reads it, so the
                    PSUM slot recycles in ~0.7us instead of ~6.5us (the
                    reciprocal-blocks-scores WAR stall)."""
                    h, pr = b // NPAIR, b % NPAIR
                    q0 = pr * 1024
                    acc_d, acc_p = acc_t.pop(b)
                    srcs = [acc_d, acc_p] + pt_tail.pop(b)
                    sums = sums_pool.tile(
                        [128, 1024], F32, tag=sums_tag, name="sums"
                    )
                    for hf in range(2):
                        hs = slice(hf * 512, (hf + 1) * 512)
                        for si, src in enumerate(srcs):
                            nc.tensor.matmul(
                                sums[:, hs], ones_bf, src[:, hs],
                                start=(si == 0), stop=(si == len(srcs) - 1),
                            )
                    ssb = smsb.tile([128, 1024], F32, tag="ssb", name="ssb")
                    nc.vector.tensor_copy(ssb, sums)
                    rc = rcp.tile([128, 1024], F32, tag="rc", name="rc")
                    nc.vector.reciprocal(rc, ssb)
                    # normalize on DVE right after the reciprocal so phase 3
                    # never waits behind the next block's Pool adds
                    nc.vector.tensor_tensor(
                        outT[:, h, q0 : q0 + 1024], avsb_t.pop(b), rc, Mult
                    )

                with tc.tile_pool(name="ps_mix", bufs=3, space="PSUM") as ps_mix:
                    for j in range(16 * NBLK + LOOK):
                        if j < 16 * NBLK:
                            b, tt = j // ST, j % ST
                            h, pr = b // NPAIR, b % NPAIR
                            q0 = pr * 1024
                            pss = ps_mix.tile(
                                [128, 1024], F32, tag="s", name="pss"
                            )
                            kslice = KT[:, tt * 128 : (tt + 1) * 128]
                            for hf in range(2):
                                nc.tensor.matmul(
                                    pss[:, hf * 512 : (hf + 1) * 512],
                                    kslice,
                                    QT[:, h, q0 + hf * 512 : q0 + (hf + 1) * 512],
                                    start=True, stop=True,
                                )
                            pss_t[j] = pss
                        jj = j - LOOK
                        if 0 <= jj < 16 * NBLK:
                            b, tt = jj // ST, jj % ST
                            h, pr = b // NPAIR, b % NPAIR
                            q0 = pr * 1024
                            pt = probs_pool.tile(
                                [128, 1024], BF16, tag="pt", name="pt"
                            )
                            nc.scalar.activation(pt, pss_t.pop(jj), Exp)
                            # denominator accumulation split, sized from
                            # measured per-op costs so each engine stays
                            # under PE's per-block time: DVE gets the init
                            # copy + 4 even tiles, Pool 6 tiles, PE the last
                            # 5 via finalize's ones-matmul
                            if tt == 0:
                                acc_d = accp.tile(
                                    [128, 1024], BF16, tag="accd", name="accd"
                                )
                                acc_p = accp.tile(
                                    [128, 1024], BF16, tag="accp", name="accp"
                                )
                                acc_t[b] = (acc_d, acc_p)
                                pt_tail[b] = []
                                nc.vector.tensor_copy(acc_d, pt)
                                nc.gpsimd.memset(acc_p, 0.0)
                                av_t[b] = ps_av.tile(
                                    [128, 1024], F32, tag="av", name="avp"
                                )
                            elif tt >= (ST - 7 if b == NBLK - 1 else ST - 5):
                                # last block hands two extra tiles to PE:
                                # there is no following block to hide the
                                # Pool adds' latency behind
                                pt_tail[b].append(pt)
                            elif tt % 2 == 0 and tt <= 8:
                                acc_d = acc_t[b][0]
                                nc.vector.tensor_tensor(acc_d, acc_d, pt, Add)
                            else:
                                acc_p = acc_t[b][1]
                                nc.gpsimd.tensor_tensor(acc_p, acc_p, pt, Add)
                            av = av_t[b]
                            for hf in range(2):
                                hs = slice(hf * 512, (hf + 1) * 512)
                                nc.tensor.matmul(
                                    av[:, hs], V[:, tt], pt[:, hs],
                                    start=(tt == 0), stop=(tt == ST - 1),
                                )
                            if tt == ST - 1:
                                # drain AV psum immediately so the next
                                # block's AV start never waits on the
                                # normalize chain
                                asb = avsb.tile(
                                    [128, 1024], F32, tag="avsb", name="asb"
                                )
                                avsb_t[b] = asb
                                av = av_t.pop(b)
                                # split halves: the next block's first AV
                                # matmul (hf=0) unblocks after the first
                                # half-copy instead of the full drain
                                nc.vector.tensor_copy(
                                    asb[:, 0:512], av[:, 0:512]
                                )
                                nc.vector.tensor_copy(
                                    asb[:, 512:1024], av[:, 512:1024]
                                )
                        # per-block tail emitted after this iteration's
                        # scores so the exp pipeline is never starved by
                        # the 14-matmul sums burst
                        if j >= 21 and (j - 21) % 16 == 0 and (j - 21) // 16 < NBLK - 1:
                            finalize((j - 21) // 16, ps_mix, "s")

                    # last block's reduction goes through the AV pool (not
                    # ps_mix), emitted before ps_mix closes so the close and
                    # phase 3's pool open overlap the final reciprocal
                    finalize(NBLK - 1, ps_av, "av")

                # ---- phase 3: output projection (transposed) ----
                otr = otd.rearrange("(o p) m -> p o m", p=128)
                with tc.tile_pool(name="ostage", bufs=4) as ostage, \
                     tc.tile_pool(name="ps_o", bufs=6, space="PSUM") as ps_o:
                    # sc outer: the first tiles only need the pr=0 (even)
                    # blocks, so P3 never waits on the last blocks' deferred
                    # normalize; output DMAs go out in et-pairs alternating
                    # between two DGE rings so the ring never backs up
                    for sc in range(SC):
                        cs = slice(sc * 512, (sc + 1) * 512)
                        for etp in range(ET // 2):
                            st = ostage.tile(
                                [128, 2, 512], F32, tag="ost", name="st"
                            )
                            for k in range(2):
                                et = etp * 2 + k
                                ps = ps_o.tile(
                                    [128, 512], F32, tag="po", name="ps"
                                )
                                for h in range(R):
                                    nc.tensor.matmul(
                                        ps,
                                        wo_sb[:, h, et * 128 : (et + 1) * 128],
                                        outT[:, h, cs],
                                        start=(h == 0), stop=(h == R - 1),
                                    )
                                nc.vector.tensor_copy(st[:, k], ps)
                            eng = (nc.sync, nc.scalar, nc.gpsimd)[etp % 3]
                            eng.dma_start(
                                otr[:, etp * 2 : etp * 2 + 2, cs], st
                            )

    _split_multi_waits(nc)
    return nc


def _prepare(x, Wq, bq, Wk, bk, Wv, bv, Wo, bo):
    """Host-side sharding: build per-core input maps."""
    x = np.asarray(x, dtype=np.float32)
    Wq = np.asarray(Wq, dtype=np.float32)
    bq = np.asarray(bq, dtype=np.float32)
    Wk = np.asarray(Wk, dtype=np.float32)
    bk = np.asarray(bk, dtype=np.float32)
    Wv = np.asarray(Wv, dtype=np.float32)
    bv = np.asarray(bv, dtype=np.float32)
    Wo = np.asarray(Wo, dtype=np.float32)

    import ml_dtypes

    BF = ml_dtypes.bfloat16
    isd = np.float32(1.0 / np.sqrt(D))
    xTs = [np.ascontiguousarray(x[b].T.astype(BF)) for b in range(B)]
    in_maps = []
    for core in range(8):
        b, g = divmod(core, G)
        in_maps.append({
            "xT": xTs[b],
            "wq": np.ascontiguousarray(
                (Wq[:, g * R * D : (g + 1) * R * D] * isd).astype(BF)
            ),
            "wk": np.ascontiguousarray(Wk[:, g * D : (g + 1) * D].astype(BF)),
            "wv": np.ascontiguousarray(Wv[:, g * D : (g + 1) * D].astype(BF)),
            "wo": np.ascontiguousarray(Wo[g * R * D : (g + 1) * R * D, :]),
            "bqv": bq[g * R * D : (g + 1) * R * D] * isd,
            "bkv": bk[g * D : (g + 1) * D],
            "bvv": bv[g * D : (g + 1) * D],
        })
    return in_maps


def _gather(results, bo):
    bo = np.asarray(bo, dtype=np.float32)
    out = np.empty((B, S, E), dtype=np.float32)
    for b in range(B):
        acc = results[b * G]["ot"].copy()
        for g in range(1, G):
            acc += results[b * G + g]["ot"]
        out[b] = acc.T + bo
    return out


def kernel(x, Wq, bq, Wk, bk, Wv, bv, Wo, bo):
    from concourse.bass_utils import run_bass_kernel_spmd

    if "nc" not in _cache:
        _cache["nc"] = _build_program()
    nc = _cache["nc"]
    in_maps = _prepare(x, Wq, bq, Wk, bk, Wv, bv, Wo, bo)
    res = run_bass_kernel_spmd(nc, in_maps, core_ids=list(range(8)))
    return _gather(res.results, bo)


# revision 43
# speedup vs baseline: 1.0128x; 1.0103x over previous
"""GQA attention kernel for 8 Trainium2 NeuronCores.

Sharding: core = (batch b, kv_group g), b in {0,1}, g in {0..3}.
Each core computes the 4 heads of one KV group for one batch and the
partial output projection for those heads; the host sums the 4 group
partials per batch.  Zero duplicated compute across cores.

v4 design (baseline was 516us):
  - P1 (QKV proj) e-outer with 6 concurrent PSUM accumulation groups;
    e-granular first-quarter DMAs spread across the SP/ACT/Pool DGE
    queues so the first matmul starts ~12us in and PE never waits on
    DMA; V transposed through a side PSUM bank inside the loop.
  - P2 (attention) software-pipelined with lookahead-2 scores in a
    3-deep PSUM rotation so PE never stalls on the ACT exp chain.
  - softmax denominators: probs written bf16; accumulation split
    across DVE (5 tiles + init copy), Pool (7 tiles), and PE
    (3 tail tiles via the final ones-matmul), sized from measured
    per-op costs so every engine stays under PE's per-block time.
  - per-block normalize (reciprocal + multiply) deferred 4 tiles into
    the next block, AV PSUM drained immediately by DVE, so no PE
    instruction ever waits on the normalize chain; the last block's
    reduction uses the AV PSUM pool so phase 3's PSUM pool opens
    without waiting on it.
  - numerics: scores/Q/K/weights stay fp32r; only probs/V/acc are
    bf16 (validated 2.1e-3 max rel err vs 2e-2 budget).
"""

import numpy as np

# problem shape (hardcoded per contract)
B, S, E = 2, 2048, 2048
H, G, D = 16, 4, 128
R = H // G          # heads per kv group = 4
KV = G * D          # 512
ST = S // 128       # 16 t-tiles
ET = E // 128       # 16 e-tiles
SC = S // 512       # 4 s-chunks
NPAIR = S // 1024   # 2 q-chunk pairs
NBLK = R * NPAIR    # 8 attention blocks per core
LOOK = 2            # scores lookahead (PSUM rotation depth - 1)

_cache = {}


def _split_multi_waits(nc, maxw=1):
    """Walrus in this container accepts only one sync-wait per
    instruction; move extra waits onto preceding same-engine NoOps."""
    from concourse import mybir

    n_split = 0
    for fn in nc.m.functions:
        for bb in fn.blocks:
            out = []
            changed = False
            for inst in bb.instructions:
                si = inst.sync_info
                waits = list(si.on_wait or []) if si is not None else []
                if len(waits) > maxw:
                    changed = True
                    n_split += 1
                    head, tail = waits[:-maxw], waits[-maxw:]
                    for j in range(0, len(head), maxw):
                        nop = mybir.InstNoOp(
                            name=f"{inst.name}-wsplit{j}", ins=[], outs=[]
                        )
                        nop.engine = inst.engine
                        nop.sync_info = mybir.SyncInfo(
                            on_wait=head[j : j + maxw], on_update=[]
                        )
                        out.append(nop)
                    si.on_wait = tail
                out.append(inst)
            if changed:
                bb.instructions = out
    return n_split


def _build_program():
    import concourse.bass as bass
    import concourse.tile as tile
    from concourse import mybir
    from concourse.masks import make_identity

    F32R = mybir.dt.float32r
    F32 = mybir.dt.float32
    BF16 = mybir.dt.bfloat16
    Exp = mybir.ActivationFunctionType.Exp
    Mult = mybir.AluOpType.mult
    Add = mybir.AluOpType.add

    nc = bass.Bass(target_bir_lowering=False)

    # x and the QKV weights arrive as bf16 (host-cast): halves input DMA
    # bytes — the DGE rings are the phase-1 constraint — at identical
    # matmul throughput (1 row/cycle for bf16 and fp32r alike)
    xT = nc.dram_tensor("xT", [E, S], BF16, kind="ExternalInput")
    wq = nc.dram_tensor("wq", [E, R * D], BF16, kind="ExternalInput")
    wk = nc.dram_tensor("wk", [E, D], BF16, kind="ExternalInput")
    wv = nc.dram_tensor("wv", [E, D], BF16, kind="ExternalInput")
    wo = nc.dram_tensor("wo", [R * D, E], F32R, kind="ExternalInput")
    bqv = nc.dram_tensor("bqv", [R * D], F32, kind="ExternalInput")
    bkv = nc.dram_tensor("bkv", [D], F32, kind="ExternalInput")
    bvv = nc.dram_tensor("bvv", [D], F32, kind="ExternalInput")
    otd = nc.dram_tensor("ot", [E, S], F32, kind="ExternalOutput")

    xTr = xT.rearrange("(o p) m -> p o m", p=128)
    wqr = wq.rearrange("(o p) m -> p o m", p=128)
    wkr = wk.rearrange("(o p) m -> p o m", p=128)
    wvr = wv.rearrange("(o p) m -> p o m", p=128)
    wor = wo.rearrange("(o p) m -> p o m", p=128)

    with tile.TileContext(nc) as tc:
        import contextlib

        with contextlib.ExitStack() as ctx:
            consts = ctx.enter_context(tc.tile_pool(name="consts", bufs=1))
            qkvt = ctx.enter_context(tc.tile_pool(name="qkvt", bufs=1))

            ident_f = consts.tile([128, 128], F32)
            make_identity(nc, ident_f)
            ident = consts.tile([128, 128], F32R)
            nc.vector.tensor_copy(ident, ident_f)
            ones_bf = consts.tile([128, 128], BF16)
            nc.gpsimd.memset(ones_bf, 1.0)
            bq_sb = consts.tile([128, R], F32)
            bk_sb = consts.tile([128, 1], F32)
            bv_sb = consts.tile([128, 1], F32)

            QT = qkvt.tile([128, R, S], F32R)    # QT[d, h, s]
            KT = qkvt.tile([128, S], F32R)       # KT[d, t]
            V = qkvt.tile([128, ST, D], BF16)    # V[t%128, tt, d]

            # ---- phase 1: QKV^T projections + V transpose ----
            with tc.tile_pool(name="vt", bufs=1) as vtpool, \
                 tc.tile_pool(name="wts", bufs=1) as wpool, \
                 tc.tile_pool(name="xts", bufs=4) as xtpool, \
                 tc.tile_pool(name="ps1", bufs=7, space="PSUM") as ps1, \
                 tc.tile_pool(name="psv", bufs=1, space="PSUM") as psv:
                VT = vtpool.tile([128, S], F32R)
                wq_sb = wpool.tile([128, ET, R * D], BF16)
                wk_sb = wpool.tile([128, ET, D], BF16)
                wv_sb = wpool.tile([128, ET, D], BF16)
                # e-granular DMAs for the first quarter so the first
                # matmuls unblock asap; remaining quarters spread over the
                # SP and ACT DGE queues so neither queue serializes >3MB
                # each DGE ring transfers serially (~7us/MB + ~1.4us fixed
                # per DMA), so order every ring by the e-index at which PE
                # first needs the data
                nc.sync.dma_start(wq_sb[:, 0:1], wqr[:, 0:1])
                nc.scalar.dma_start(wk_sb[:, 0:4], wkr[:, 0:4])
                nc.scalar.dma_start(wv_sb[:, 0:4], wvr[:, 0:4])
                nc.sync.dma_start(wq_sb[:, 1:4], wqr[:, 1:4])
                nc.sync.dma_start(wq_sb[:, 4:8], wqr[:, 4:8])
                nc.scalar.dma_start(wq_sb[:, 8:12], wqr[:, 8:12])
                nc.sync.dma_start(wk_sb[:, 4:10], wkr[:, 4:10])
                nc.sync.dma_start(wv_sb[:, 4:10], wvr[:, 4:10])
                nc.scalar.dma_start(wq_sb[:, 12:16], wqr[:, 12:16])
                nc.sync.dma_start(wk_sb[:, 10:ET], wkr[:, 10:ET])
                nc.sync.dma_start(wv_sb[:, 10:ET], wvr[:, 10:ET])
                # biases are tiny and needed late; issue after the weights
                nc.sync.dma_start(bq_sb, bqv.rearrange("(o p) -> p o", p=128))
                nc.sync.dma_start(bk_sb, bkv.rearrange("(o p) -> p o", p=128))
                nc.sync.dma_start(bv_sb, bvv.rearrange("(o p) -> p o", p=128))

                def transposes(sc):
                    tps = psv.tile([128, 512], F32R, tag="pv", name="tps")
                    for i in range(4):
                        tt = sc * 4 + i
                        nc.tensor.transpose(
                            tps[:, i * 128 : (i + 1) * 128],
                            VT[:, tt * 128 : (tt + 1) * 128],
                            ident,
                        )
                    for i in range(4):
                        nc.vector.tensor_copy(
                            V[:, sc * 4 + i], tps[:, i * 128 : (i + 1) * 128]
                        )

                for sc in range(SC):
                    cs = slice(sc * 512, (sc + 1) * 512)
                    po = [ps1.tile([128, 512], F32, tag="p1", name="po")
                          for _ in range(R + 2)]
                    for eq in range(4):
                        xq = xtpool.tile([128, 4, 512], BF16, tag="xt")
                        if sc == 0 and eq == 0:
                            # e-granular so the first matmul starts early
                            for i in range(4):
                                nc.gpsimd.dma_start(
                                    xq[:, i : i + 1], xTr[:, i : i + 1, cs]
                                )
                        elif sc == 0:
                            # pair-granular through the rest of the first
                            # chunk: the ring can't stay ahead of PE with
                            # full quarters this early
                            for i in (0, 2):
                                e0 = eq * 4 + i
                                nc.gpsimd.dma_start(
                                    xq[:, i : i + 2], xTr[:, e0 : e0 + 2, cs]
                                )
                        else:
                            nc.gpsimd.dma_start(
                                xq, xTr[:, eq * 4 : eq * 4 + 4, cs]
                            )
                        for i in range(4):
                            e = eq * 4 + i
                            for ot in range(R + 2):
                                if ot < R:
                                    lhsT = wq_sb[:, e, ot * 128 : (ot + 1) * 128]
                                elif ot == R:
                                    lhsT = wk_sb[:, e]
                                else:
                                    lhsT = wv_sb[:, e]
                                nc.tensor.matmul(
                                    po[ot], lhsT, xq[:, i],
                                    start=(e == 0), stop=(e == ET - 1),
                                )
                        if eq == 1 and sc > 0:
                            # previous chunk's V rows are long since
                            # drained; transpose them here so PE never
                            # waits on the ACT drain queue
                            transposes(sc - 1)
                    # drains; for the last chunk emit V first so its
                    # transposes (right below) wait minimally
                    drains = [(VT[:, cs], po[R + 1], bv_sb[:, 0:1]),
                              (KT[:, cs], po[R], bk_sb[:, 0:1])]
                    qdr = [(QT[:, ot, cs], po[ot], bq_sb[:, ot : ot + 1])
                           for ot in range(R)]
                    order = drains + qdr if sc == SC - 1 else qdr + drains[::-1]
                    for dst, src, bias in order:
                        nc.scalar.add(dst, src, bias)
                transposes(SC - 1)

            # ---- phase 2: attention, software-pipelined ----
            p23 = ctx.enter_context(tc.tile_pool(name="p23", bufs=1))
            outT = p23.tile([128, R, S], F32R)  # normalized attn outT[d, h, s]
            wo_sb = p23.tile([128, R, E], F32R)
            for q in range(4):
                nc.sync.dma_start(wo_sb[:, q], wor[:, q])

            with tc.tile_pool(name="ps_av", bufs=1, space="PSUM") as ps_av, \
                 tc.tile_pool(name="probs", bufs=10) as probs_pool, \
                 tc.tile_pool(name="accp", bufs=2) as accp, \
                 tc.tile_pool(name="avsb", bufs=2) as avsb, \
                 tc.tile_pool(name="smsb", bufs=2) as smsb, \
                 tc.tile_pool(name="rcp", bufs=2) as rcp:

                pss_t = {}   # j -> scores psum tile
                acc_t = {}   # blk -> (dve_acc, pool_acc) bf16 accumulators
                av_t = {}    # blk -> AV psum tile
                avsb_t = {}  # blk -> AV sbuf drain tile
                pt_tail = {}  # blk -> tail prob tiles summed directly on PE

                def finalize(b, sums_pool, sums_tag):
                    """Deferred per-block tail: reduce the prob accumulators
                    (+ the tail prob tiles, summed directly on PE to offload
                    DVE/Pool), reciprocal, and normalize into outT.  Runs 5
                    tiles into the next block so nothing on PE ever waits
                    for it.  The sums PSUM tile is drained to SBUF by a fast
                    DVE copy before the slow reciprocal reads it, so the
                    PSUM slot recycles in ~0.7us instead of ~6.5us (the
                    reciprocal-blocks-scores WAR stall)."""
                    h, pr = b // NPAIR, b % NPAIR
                    q0 = pr * 1024
                    acc_d, acc_p = acc_t.pop(b)
                    srcs = [acc_d, acc_p] + pt_tail.pop(b)
                    sums = sums_pool.tile(
                        [128, 1024], F32, tag=sums_tag, name="sums"
                    )
                    for hf in range(2):
                        hs = slice(hf * 512, (hf + 1) * 512)
                        for si, src in enumerate(srcs):
                            nc.tensor.matmul(
                                sums[:, hs], ones_bf, src[:, hs],
                                start=(si == 0), stop=(si == len(srcs) - 1),
                            )
                    ssb = smsb.tile([128, 1024], F32, tag="ssb", name="ssb")
                    nc.vector.tensor_copy(ssb, sums)
                    rc = rcp.tile([128, 1024], F32, tag="rc", name="rc")
                    nc.vector.reciprocal(rc, ssb)
                    # normalize on DVE right after the reciprocal so phase 3
                    # never waits behind the next block's Pool adds
                    nc.vector.tensor_tensor(
                        outT[:, h, q0 : q0 + 1024], avsb_t.pop(b), rc, Mult
                    )

                with tc.tile_pool(name="ps_mix", bufs=3, space="PSUM") as ps_mix:
                    for j in range(16 * NBLK + LOOK):
                        if j < 16 * NBLK:
                            b, tt = j // ST, j % ST
                            h, pr = b // NPAIR, b % NPAIR
                            q0 = pr * 1024
                            pss = ps_mix.tile(
                                [128, 1024], F32, tag="s", name="pss"
                            )
                            kslice = KT[:, tt * 128 : (tt + 1) * 128]
                            for hf in range(2):
                                nc.tensor.matmul(
                                    pss[:, hf * 512 : (hf + 1) * 512],
                                    kslice,
                                    QT[:, h, q0 + hf * 512 : q0 + (hf + 1) * 512],
                                    start=True, stop=True,
                                )
                            pss_t[j] = pss
                        jj = j - LOOK
                        if 0 <= jj < 16 * NBLK:
                            b, tt = jj // ST, jj % ST
                            h, pr = b // NPAIR, b % NPAIR
                            q0 = pr * 1024
                            pt = probs_pool.tile(
                                [128, 1024], BF16, tag="pt", name="pt"
                            )
                            nc.scalar.activation(pt, pss_t.pop(jj), Exp)
                            # denominator accumulation split, sized from
                            # measured per-op costs so each engine stays
                            # under PE's per-block time: DVE gets the init
                            # copy + 4 even tiles, Pool 6 tiles, PE the last
                            # 5 via finalize's ones-matmul
                            if tt == 0:
                                acc_d = accp.tile(
                                    [128, 1024], BF16, tag="accd", name="accd"
                                )
                                acc_p = accp.tile(
                                    [128, 1024], BF16, tag="accp", name="accp"
                                )
                                acc_t[b] = (acc_d, acc_p)
                                pt_tail[b] = []
                                nc.vector.tensor_copy(acc_d, pt)
                                nc.gpsimd.memset(acc_p, 0.0)
                                av_t[b] = ps_av.tile(
                                    [128, 1024], F32, tag="av", name="avp"
                                )
                            elif tt >= (ST - 7 if b == NBLK - 1 else ST - 5):
                                # last block hands two extra tiles to PE:
                                # there is no following block to hide the
                                # Pool adds' latency behind
                                pt_tail[b].append(pt)
                            elif tt % 2 == 0 and tt <= 8:
                                acc_d = acc_t[b][0]
                                nc.vector.tensor_tensor(acc_d, acc_d, pt, Add)
                            else:
                                acc_p = acc_t[b][1]
                                nc.gpsimd.tensor_tensor(acc_p, acc_p, pt, Add)
                            av = av_t[b]
                            for hf in range(2):
                                hs = slice(hf * 512, (hf + 1) * 512)
                                nc.tensor.matmul(
                                    av[:, hs], V[:, tt], pt[:, hs],
                                    start=(tt == 0), stop=(tt == ST - 1),
                                )
                            if tt == ST - 1:
                                # drain AV psum immediately so the next
                                # block's AV start never waits on the
                                # normalize chain
                                asb = avsb.tile(
                                    [128, 1024], F32, tag="avsb", name="asb"
                                )
                                avsb_t[b] = asb
                                av = av_t.pop(b)
                                # split halves: the next block's first AV
                                # matmul (hf=0) unblocks after the first
                                # half-copy instead of the full drain
                                nc.vector.tensor_copy(
                                    asb[:, 0:512], av[:, 0:512]
                                )
                                nc.vector.tensor_copy(
                                    asb[:, 512:1024], av[:, 512:1024]
                                )
                        # per-block tail emitted after this iteration's
                        # scores so the exp pipeline is never starved by
                        # the 14-matmul sums burst
                        if j >= 21 and (j - 21) % 16 == 0 and (j - 21) // 16 < NBLK - 1:
                            finalize((j - 21) // 16, ps_mix, "s")

                    # last block's reduction goes through the AV pool (not
                    # ps_mix), emitted before ps_mix closes so the close and
                    # phase 3's pool open overlap the final reciprocal
                    finalize(NBLK - 1, ps_av, "av")

                # ---- phase 3: output projection (transposed) ----
                otr = otd.rearrange("(o p) m -> p o m", p=128)
                with tc.tile_pool(name="ostage", bufs=4) as ostage, \
                     tc.tile_pool(name="ps_o", bufs=6, space="PSUM") as ps_o:
                    # sc outer: the first tiles only need the pr=0 (even)
                    # blocks, so P3 never waits on the last blocks' deferred
                    # normalize; output DMAs go out in et-pairs alternating
                    # between two DGE rings so the ring never backs up
                    for sc in range(SC):
                        cs = slice(sc * 512, (sc + 1) * 512)
                        for etp in range(ET // 2):
                            st = ostage.tile(
                                [128, 2, 512], F32, tag="ost", name="st"
                            )
                            for k in range(2):
                                et = etp * 2 + k
                                ps = ps_o.tile(
                                    [128, 512], F32, tag="po", name="ps"
                                )
                                for h in range(R):
                                    nc.tensor.matmul(
                                        ps,
                                        wo_sb[:, h, et * 128 : (et + 1) * 128],
                                        outT[:, h, cs],
                                        start=(h == 0), stop=(h == R - 1),
                                    )
                                nc.vector.tensor_copy(st[:, k], ps)
                            eng = (nc.sync, nc.scalar, nc.gpsimd)[etp % 3]
                            eng.dma_start(
                                otr[:, etp * 2 : etp * 2 + 2, cs], st
                            )

    _split_multi_waits(nc)
    return nc


def _prepare(x, Wq, bq, Wk, bk, Wv, bv, Wo, bo):
    """Host-side sharding: build per-core input maps."""
    x = np.asarray(x, dtype=np.float32)
    Wq = np.asarray(Wq, dtype=np.float32)
    bq = np.asarray(bq, dtype=np.float32)
    Wk = np.asarray(Wk, dtype=np.float32)
    bk = np.asarray(bk, dtype=np.float32)
    Wv = np.asarray(Wv, dtype=np.float32)
    bv = np.asarray(bv, dtype=np.float32)
    Wo = np.asarray(Wo, dtype=np.float32)

    import ml_dtypes

    BF = ml_dtypes.bfloat16
    isd = np.float32(1.0 / np.sqrt(D))
    xTs = [np.ascontiguousarray(x[b].T.astype(BF)) for b in range(B)]
    in_maps = []
    for core in range(8):
        b, g = divmod(core, G)
        in_maps.append({
            "xT": xTs[b],
            "wq": np.ascontiguousarray(
                (Wq[:, g * R * D : (g + 1) * R * D] * isd).astype(BF)
            ),
            "wk": np.ascontiguousarray(Wk[:, g * D : (g + 1) * D].astype(BF)),
            "wv": np.ascontiguousarray(Wv[:, g * D : (g + 1) * D].astype(BF)),
            "wo": np.ascontiguousarray(Wo[g * R * D : (g + 1) * R * D, :]),
            "bqv": bq[g * R * D : (g + 1) * R * D] * isd,
            "bkv": bk[g * D : (g + 1) * D],
            "bvv": bv[g * D : (g + 1) * D],
        })
    return in_maps


def _gather(results, bo):
    bo = np.asarray(bo, dtype=np.float32)
    out = np.empty((B, S, E), dtype=np.float32)
    for b in range(B):
        acc = results[b * G]["ot"].copy()
        for g in range(1, G):
            acc += results[b * G + g]["ot"]
        out[b] = acc.T + bo
    return out


def kernel(x, Wq, bq, Wk, bk, Wv, bv, Wo, bo):
    from concourse.bass_utils import run_bass_kernel_spmd

    if "nc" not in _cache:
        _cache["nc"] = _build_program()
    nc = _cache["nc"]
    in_maps = _prepare(x, Wq, bq, Wk, bk, Wv, bv, Wo, bo)
    res = run_bass_kernel_spmd(nc, in_maps, core_ids=list(range(8)))
    return _gather(res.results, bo)
